# revision 26
# baseline (speedup 1.0000x reference)
"""Causal self-attention (lit-gpt style, partial RoPE) on 8 Trainium2 NeuronCores.

Sharding: tensor-parallel over heads. Each core owns 4 of the 32 heads
end-to-end (QKV projection, RoPE, causal SDPA, and the K-split slice of the
output projection). Each core emits a partial (T, 4096) output; the host sums
the 8 partials (mathematically the all-reduce) and applies the bias folds.

Device-side compute dtype: bf16 matmul inputs with fp32 PSUM accumulation.

Model shapes (hardcoded): B=1, T=2048, C=4096, H=32, D=128, R=32 (rope),
rope base 10000.

v2 scheduling notes (vs the 699us v1):
 - QKV q/k features are processed in two groups of 4 so PSUM cycles
   4(A)+4(B)+4(V) banks; each group's evictions hide under the next group's
   matmuls instead of stalling the PE at chunk boundaries.
 - PSUM evictions go to the scalar engine (bias is all-zero for this problem;
   a nonzero-bias build falls back to DVE tensor_scalar_add).
 - x / w DMAs are batched 4 k-tiles per descriptor to cut sync-queue dispatch.
 - softmax reciprocal uses the 1-op DVE approx (~18 bits, plenty for bf16
   data) instead of the 3.3us InstReciprocal, removing the head-pair boundary
   stall that caused HAM clock-gate oscillation in SDPA.
"""

import sys
from contextlib import ExitStack

sys.path.insert(0, "/opt/trn_rl_repo")

import numpy as np
import ml_dtypes

import concourse.bass as bass
import concourse.bacc as bacc
import concourse.tile as tile
from concourse import mybir
from concourse import bass_utils

BF16 = ml_dtypes.bfloat16

T = 2048
C = 4096
H = 32
D = 128
R = 32
ROPE_BASE = 10000.0
N_CORES = 8
H_LOC = H // N_CORES          # 4 heads per core
KT = C // 128                 # 32 contraction tiles
KB = KT // 4                  # 8 batched-DMA groups of 4 k-tiles
NCH = T // 512                # 4 token chunks of 512
FQK = 2 * H_LOC               # 8 q/k feature tiles: f=2h -> q_h, f=2h+1 -> k_h
SCALE = 1.0 / float(np.sqrt(D))

# set by test.py to capture an NTFF profile; harness leaves False
TRACE = False
LAST_EXEC_NS = None
LAST_RESULTS = None

_CACHE = {}


def _build_program(bias_zero):
    """Build + compile the single-program SPMD Bass module (same code on all
    8 cores; per-core weights arrive via in_maps)."""
    nc = bacc.Bacc(
        "TRN2",
        target_bir_lowering=False,
        debug=False,
        enable_asserts=False,
        num_devices=N_CORES,
    )
    bf = mybir.dt.bfloat16
    f32 = mybir.dt.float32

    # [p, kt, t] layouts so one DMA covers several k-tiles
    xT_d = nc.dram_tensor("xT", (128, KT, T), bf, kind="ExternalInput").ap()
    wqkT_d = nc.dram_tensor("wqkT", (128, KT, FQK * 128), bf, kind="ExternalInput").ap()
    wvT_d = nc.dram_tensor("wvT", (128, KT, H_LOC * 128), bf, kind="ExternalInput").ap()
    wpT_d = nc.dram_tensor("wpT", (128, H_LOC, C), bf, kind="ExternalInput").ap()
    bqk_d = nc.dram_tensor("bqk", (128, FQK), f32, kind="ExternalInput").ap()
    cos_d = nc.dram_tensor("cosP", (R, T), bf, kind="ExternalInput").ap()
    sin_d = nc.dram_tensor("sinP", (R, T), bf, kind="ExternalInput").ap()
    mask_d = nc.dram_tensor("maskP", (4, 128, 512), bf, kind="ExternalInput").ap()
    maskb_d = nc.dram_tensor("maskB", (4, 128, 512), bf, kind="ExternalInput").ap()
    ident_d = nc.dram_tensor("identP", (128, 128), bf, kind="ExternalInput").ap()
    out_d = nc.dram_tensor("out", (T, C), bf, kind="ExternalOutput").ap()

    with tile.TileContext(nc) as tc:
        _emit(nc, tc, xT_d, wqkT_d, wvT_d, wpT_d, bqk_d, cos_d, sin_d, mask_d,
              maskb_d, ident_d, out_d, bias_zero)

    nc.compile()
    return nc


def _emit(nc, tc, xT_d, wqkT_d, wvT_d, wpT_d, bqk_d, cos_d, sin_d, mask_d,
          maskb_d, ident_d, out_d, bias_zero):
    bf = mybir.dt.bfloat16
    f32 = mybir.dt.float32

    # ---- persistent SBUF tensors (created before the working pools so that
    # pool release order stays LIFO: pools close first, singles after) -------
    frees = []

    def single(shape, dtype, name):
        t, fr = tc.tile(shape, dtype, name=name)
        frees.append(fr)
        return t

    qkT = single([128, FQK, T], bf, "qkT")          # Q^T/K^T: [d, f, t]
    vN = single([128, T // 128, H_LOC * 128], bf, "vN")  # V: [t%128, t//128, dv]
    yT = single([128, H_LOC, T], bf, "yT")          # normalized O^T per head
    ones = single([128, 128], bf, "ones")
    nc.vector.memset(ones, 1.0)
    # constants ride the gpsimd (SWDGE) queues so the sync queues can start
    # streaming the first x/w tiles immediately at kernel start
    bqk_sb = single([128, FQK], f32, "bqk_sb")
    if not bias_zero:
        nc.gpsimd.dma_start(out=bqk_sb, in_=bqk_d)
    cos_sb = single([R, T], bf, "cos_sb")
    sin_sb = single([R, T], bf, "sin_sb")
    nc.gpsimd.dma_start(out=cos_sb, in_=cos_d)
    nc.gpsimd.dma_start(out=sin_sb, in_=sin_d)
    # diagonal causal mask tiles: mask_r[jj, ii] = 1.0 if ii >= jj + 128*r
    maskt = single([128, 4, 512], bf, "maskt")
    for r in range(4):
        nc.gpsimd.dma_start(out=maskt[:, r, :], in_=mask_d[r])
    masks = [maskt[:, r, :] for r in range(4)]
    # additive variant (-30000 where masked) for chunk 0, where EVERY tile is
    # masked: folded into the score PSUM by an identity-lhsT matmul so the
    # DVE never sits on chunk 0's critical exp->mask->PV chain
    maskbt = single([128, 4, 512], bf, "maskbt")
    for r in range(4):
        nc.gpsimd.dma_start(out=maskbt[:, r, :], in_=maskb_d[r])
    ident = single([128, 128], bf, "ident")
    nc.gpsimd.dma_start(out=ident, in_=ident_d)
    # v weights are chunk-independent: keep resident instead of re-streaming
    wv_res = single([128, KT, 512], bf, "wv_res")

    with ExitStack() as ctx:
        xpool = ctx.enter_context(tc.tile_pool(name="xp", bufs=9))
        wqpool = ctx.enter_context(tc.tile_pool(name="wq", bufs=6))
        attpool = ctx.enter_context(tc.tile_pool(name="att", bufs=6))
        ropepool = ctx.enter_context(tc.tile_pool(name="rope", bufs=2))
        recippool = ctx.enter_context(tc.tile_pool(name="recip", bufs=2))
        stagepool = ctx.enter_context(tc.tile_pool(name="stage", bufs=3))
        wppool = ctx.enter_context(tc.tile_pool(name="wp", bufs=3))
        psum = ctx.enter_context(tc.tile_pool(name="psum", bufs=8, space="PSUM"))

        # PE warm-up: throwaway matmuls issued while the first weight and
        # activation DMAs are in flight, to lift the PE HAM clock gate to 8/8
        # before the real work arrives.
        warm = psum.tile([128, 128], f32, name="warm", tag="ps")
        for _ in range(96):
            nc.tensor.matmul(warm, lhsT=ones, rhs=ones, start=True, stop=True)

        def emit_rope(f, c):
            # q'[0:16]  = q[0:16]*cos - q[16:32]*sin
            # q'[16:32] = q[16:32]*cos + q[0:16]*sin
            # sin_sb rows 0..15 hold -sin, rows 16..31 hold +sin (host-folded).
            cs = slice(c * 512, (c + 1) * 512)
            rows = qkT[0:R, f, cs]
            swap = ropepool.tile([R, 512], bf, name=f"swap{f}_{c}", tag="swap")
            # gpsimd (SWDGE) queues: keeps these small SBUF->SBUF copies off
            # the sync queues that stream the main weight/activation tiles
            nc.gpsimd.dma_start(out=swap[0:16, :], in_=qkT[16:32, f, cs])
            nc.gpsimd.dma_start(out=swap[16:32, :], in_=qkT[0:16, f, cs])
            t1 = ropepool.tile([R, 512], bf, name=f"t1_{f}_{c}", tag="t1")
            nc.vector.tensor_mul(t1, swap, sin_sb[:, cs])
            nc.vector.tensor_mul(rows, rows, cos_sb[:, cs])
            nc.vector.tensor_add(rows, rows, t1)

        # ---- phase 1: QKV projection -------------------------------------
        # x batch tiles are shared by the A / B / V passes of a chunk.
        x_tiles = {}

        def emit_qk_group(c, g):
            # features g*4 .. g*4+3 in transposed layout (feature-major)
            pss = [
                psum.tile([128, 512], f32, name=f"qk_ps{c}_{g}_{i}", tag="ps")
                for i in range(4)
            ]
            for kb in range(KB):
                wb = wqpool.tile([128, 4, 512], bf, name=f"w{c}_{g}_{kb}", tag="wq")
                nc.sync.dma_start(
                    out=wb,
                    in_=wqkT_d[:, kb * 4 : (kb + 1) * 4, g * 512 : (g + 1) * 512],
                )
                xb = x_tiles[kb]
                for k4 in range(4):
                    for f in range(4):
                        nc.tensor.matmul(
                            pss[f],
                            lhsT=wb[:, k4, f * 128 : (f + 1) * 128],
                            rhs=xb[:, k4, :],
                            start=(kb == 0 and k4 == 0),
                            stop=(kb == KB - 1 and k4 == 3),
                        )
            last = c == NCH - 1
            for f in range(4):
                ff = g * 4 + f
                dst = qkT[:, ff, c * 512 : (c + 1) * 512]
                if not bias_zero:
                    nc.vector.tensor_scalar_add(dst, pss[f], bqk_sb[:, ff : ff + 1])
                elif last and g == 1:
                    # last chunk's B group: evict on DVE so the ACT queue is
                    # clear for chunk 0's first SDPA exps right after
                    nc.vector.tensor_copy(dst, pss[f])
                else:
                    nc.scalar.copy(dst, pss[f])
            if not last:
                for f in range(4):
                    emit_rope(g * 4 + f, c)
            # last chunk's rope is deferred into the SDPA phase (its k/q are
            # first consumed by the LAST SDPA chunk, ~200us later) so its DVE
            # ops don't queue ahead of chunk 0's epilogues

        def emit_v(c):
            # v in natural layout (token-major). Runs FIRST in each chunk: it
            # only streams x (weights are resident), so the qk weight streams
            # for A/B prefetch during its low-bandwidth window.
            psv = [
                psum.tile([128, 512], f32, name=f"v_ps{c}_{i}", tag="ps")
                for i in range(4)
            ]
            for kb in range(KB):
                xb = xpool.tile([128, 4, 512], bf, name=f"x{c}_{kb}", tag="x")
                nc.sync.dma_start(
                    out=xb,
                    in_=xT_d[:, kb * 4 : (kb + 1) * 4, c * 512 : (c + 1) * 512],
                )
                x_tiles[kb] = xb
                if c == 0:
                    nc.sync.dma_start(
                        out=wv_res[:, kb * 4 : (kb + 1) * 4, :],
                        in_=wvT_d[:, kb * 4 : (kb + 1) * 4, :],
                    )
                for k4 in range(4):
                    kt = kb * 4 + k4
                    for tt in range(4):
                        nc.tensor.matmul(
                            psv[tt],
                            lhsT=xb[:, k4, tt * 128 : (tt + 1) * 128],
                            rhs=wv_res[:, kt, :],
                            start=(kt == 0),
                            stop=(kt == KT - 1),
                        )
            for tt in range(4):
                nc.scalar.copy(vN[:, c * 4 + tt, :], psv[tt])

        for c in range(NCH):
            emit_v(c)
            emit_qk_group(c, 0)
            emit_qk_group(c, 1)

        # ---- phase 3+4: causal SDPA (chunk-outer, head-inner) with the
        # output projection for each finished chunk interleaved, keeping PE
        # fed while the scalar engine works on the next chunk's exps --------
        def proj_block(c, nchs):
            # output projection for the token tiles of chunk c, nch columns
            for nch in nchs:
                wp = wppool.tile([128, H_LOC, 512], bf, name=f"wp{c}_{nch}", tag="wp")
                nc.sync.dma_start(
                    out=wp, in_=wpT_d[:, :, nch * 512 : (nch + 1) * 512]
                )
                for tl in range(4):
                    tt = c * 4 + tl
                    pp = psum.tile([128, 512], f32, name=f"pp{c}_{nch}_{tl}", tag="ps")
                    for h in range(H_LOC):
                        nc.tensor.matmul(
                            pp,
                            lhsT=yT[:, h, tt * 128 : (tt + 1) * 128],
                            rhs=wp[:, h, :],
                            start=(h == 0),
                            stop=(h == H_LOC - 1),
                        )
                    st = stagepool.tile(
                        [128, 512], bf, name=f"st{c}_{nch}_{tl}", tag="st"
                    )
                    # alternate eviction between DVE and ACT: keeps the DVE
                    # FIFO short so the SDPA-critical mask/normalize ops
                    # behind it aren't delayed by bulk projection copies
                    if tl % 2 == 0:
                        nc.vector.tensor_copy(st, pp)
                    else:
                        nc.scalar.copy(st, pp)
                    nc.gpsimd.dma_start(
                        out=out_d[
                            tt * 128 : (tt + 1) * 128, nch * 512 : (nch + 1) * 512
                        ],
                        in_=st,
                    )

        for c in range(NCH):
            if c == 1:
                # deferred rope for the last QKV chunk (consumed only by the
                # last SDPA chunk): its DVE/gpsimd work hides under c1's
                # matmul stream instead of stalling chunk 0's SDPA
                for f in range(FQK):
                    emit_rope(f, NCH - 1)
            njt = 4 * (c + 1)  # causal: key tiles 0 .. 4c+3
            for hp in range(H_LOC // 2):
                # previous chunk's projection matmuls are drip-fed INTO the
                # jt loop below so the PE always has independent work while
                # the exp->mask->PV chain of the current tiles is in flight
                pending = list(range(4 * hp, 4 * hp + 4)) if c > 0 else []
                hh = (2 * hp, 2 * hp + 1)
                o_ps = {}
                d_ps = {}
                # interleave two heads so PE always has an independent matmul
                # while the scalar engine works on the other head's exp
                for jt in range(njt):
                    for h in hh:
                        s_ps = psum.tile(
                            [128, 512], f32, name=f"s_ps{h}_{c}_{jt}", tag="ps"
                        )
                        nc.tensor.matmul(
                            s_ps,
                            lhsT=qkT[:, 2 * h + 1, jt * 128 : (jt + 1) * 128],
                            rhs=qkT[:, 2 * h, c * 512 : (c + 1) * 512],
                            start=True,
                            stop=(c != 0),
                        )
                        if c == 0:
                            # additive causal mask via identity matmul: keeps
                            # chunk 0 (every tile masked) off the DVE
                            nc.tensor.matmul(
                                s_ps,
                                lhsT=ident,
                                rhs=maskbt[:, jt, :],
                                start=False,
                                stop=True,
                            )
                        att = attpool.tile(
                            [128, 512], bf, name=f"att{h}_{c}_{jt}", tag="att"
                        )
                        nc.scalar.activation(
                            out=att,
                            in_=s_ps,
                            func=mybir.ActivationFunctionType.Exp,
                            scale=SCALE,
                        )
                        if jt == 0:
                            # allocate AFTER the first score tiles so a new
                            # head-pair's scores never wait on the previous
                            # pair's o/d slots (freed by the DVE normalize)
                            o_ps[h] = psum.tile(
                                [128, 512], f32, name=f"o_ps{h}_{c}", tag="ps"
                            )
                            d_ps[h] = psum.tile(
                                [128, 512], f32, name=f"d_ps{h}_{c}", tag="ps"
                            )
                        r = jt - 4 * c
                        if r >= 0 and c != 0:
                            nc.vector.tensor_mul(att, att, masks[r])
                        nc.tensor.matmul(
                            d_ps[h],
                            lhsT=ones,
                            rhs=att,
                            start=(jt == 0),
                            stop=(jt == njt - 1),
                        )
                        nc.tensor.matmul(
                            o_ps[h],
                            lhsT=vN[:, jt, h * 128 : (h + 1) * 128],
                            rhs=att,
                            start=(jt == 0),
                            stop=(jt == njt - 1),
                        )
                    if pending and h == hh[1]:
                        # one proj column per jt, starting immediately: the
                        # pending list always drains mid-loop so no burst of
                        # evictions lands on the ACT/DVE queues at the
                        # head-pair boundary right when the next pair's exp
                        # needs them
                        proj_block(c - 1, [pending.pop(0)])
                for nch in pending:
                    proj_block(c - 1, [nch])
                for h in hh:
                    rec = recippool.tile([128, 512], f32, name=f"rec{h}_{c}", tag="rec")
                    nc.vector.reciprocal_approx_fast(rec, d_ps[h])
                    nc.vector.tensor_mul(yT[:, h, c * 512 : (c + 1) * 512], o_ps[h], rec)
        proj_block(NCH - 1, range(C // 512))

    for fr in reversed(frees):
        fr()


def _rope_tables():
    theta = 1.0 / (ROPE_BASE ** (np.arange(0, R, 2, dtype=np.float64) / R))  # (16,)
    ang = np.outer(np.arange(T, dtype=np.float64), theta)  # (T, 16)
    cos = np.cos(ang).T  # (16, T)
    sin = np.sin(ang).T
    cosP = np.concatenate([cos, cos], axis=0)  # (32, T)
    sinP = np.concatenate([-sin, sin], axis=0)
    return np.ascontiguousarray(cosP).astype(BF16), np.ascontiguousarray(sinP).astype(BF16)


def _to_p_kt(a):
    """(rows, cols) -> (128, rows//128, cols): row r = [kt*128 + p]."""
    rows, cols = a.shape
    return np.ascontiguousarray(
        a.reshape(rows // 128, 128, cols).transpose(1, 0, 2)
    )


def kernel(x, w_attn, b_attn, w_proj, b_proj):
    x = np.asarray(x, dtype=np.float32)
    w_attn = np.asarray(w_attn, dtype=np.float32)
    b_attn = np.asarray(b_attn, dtype=np.float32)
    w_proj = np.asarray(w_proj, dtype=np.float32)
    b_proj = np.asarray(b_proj, dtype=np.float32)
    B = x.shape[0]
    assert (B, x.shape[1], x.shape[2]) == (1, T, C)

    bias_zero = bool(np.all(b_attn.reshape(H, 3, D)[:, :2, :] == 0.0))
    key = ("nc", bias_zero)
    if key not in _CACHE:
        _CACHE[key] = _build_program(bias_zero)
    nc = _CACHE[key]

    xT = _to_p_kt(x[0].T.astype(BF16))  # (128, 32, T)
    cosP, sinP = _rope_tables()
    # diagonal causal mask tiles: maskP[r, jj, ii] = 1.0 iff ii >= jj + 128*r
    jj = np.arange(128)[None, :, None]
    ii = np.arange(512)[None, None, :]
    rr = (128 * np.arange(4))[:, None, None]
    keep = ii >= jj + rr
    maskP = keep.astype(BF16)  # (4, 128, 512)
    maskB = np.where(keep, 0.0, -30000.0).astype(BF16)  # additive variant
    identP = np.eye(128, dtype=np.float32).astype(BF16)

    # w_attn rows per head h: [q (128), k (128), v (128)] at offset h*384
    wa = w_attn.reshape(H, 3, D, C)
    ba = b_attn.reshape(H, 3, D)
    in_maps = []
    for core in range(N_CORES):
        hs = range(core * H_LOC, (core + 1) * H_LOC)
        qk_rows = np.concatenate(
            [wa[h, t] for h in hs for t in (0, 1)], axis=0
        )  # (1024, C)  order: q_h0, k_h0, q_h1, k_h1, ...
        v_rows = np.concatenate([wa[h, 2] for h in hs], axis=0)  # (512, C)
        wqkT = _to_p_kt(qk_rows.T.astype(BF16))  # (128, 32, 1024)
        wvT = _to_p_kt(v_rows.T.astype(BF16))  # (128, 32, 512)
        wpT = _to_p_kt(
            w_proj[:, core * 512 : (core + 1) * 512].T.astype(BF16)
        )  # (128, 4, C)
        bqk = np.ascontiguousarray(
            np.stack([ba[h, t] for h in hs for t in (0, 1)], axis=0).T
        ).astype(np.float32)  # (128, 8)
        in_maps.append(
            dict(
                xT=xT, wqkT=wqkT, wvT=wvT, wpT=wpT, bqk=bqk,
                cosP=cosP, sinP=sinP, maskP=maskP, maskB=maskB, identP=identP,
            )
        )

    res = bass_utils.run_bass_kernel_spmd(
        nc, in_maps, core_ids=list(range(N_CORES)), trace=TRACE
    )
    global LAST_EXEC_NS, LAST_RESULTS
    LAST_EXEC_NS = res.exec_time_ns
    LAST_RESULTS = res

    out = np.zeros((T, C), dtype=np.float32)
    for core in range(N_CORES):
        out += res.results[core]["out"]

    # bias folds: q/k biases were applied on device; the v bias adds exactly
    # b_v to every y row (softmax rows sum to 1), so it folds into the output
    # bias along with b_proj.
    b_v = ba[:, 2, :].reshape(-1)  # (4096,)
    out += (w_proj @ b_v + b_proj)[None, :]
    return out.reshape(B, T, C).astype(np.float32)


# revision 27
# speedup vs baseline: 1.0359x; 1.0359x over previous
"""Causal self-attention (lit-gpt style, partial RoPE) on 8 Trainium2 NeuronCores.

Sharding: tensor-parallel over heads. Each core owns 4 of the 32 heads
end-to-end (QKV projection, RoPE, causal SDPA, and the K-split slice of the
output projection). Each core emits a partial (T, 4096) output; the host sums
the 8 partials (mathematically the all-reduce) and applies the bias folds.

Device-side compute dtype: bf16 matmul inputs with fp32 PSUM accumulation.

Model shapes (hardcoded): B=1, T=2048, C=4096, H=32, D=128, R=32 (rope),
rope base 10000.

v2 scheduling notes (vs the 699us v1):
 - QKV q/k features are processed in two groups of 4 so PSUM cycles
   4(A)+4(B)+4(V) banks; each group's evictions hide under the next group's
   matmuls instead of stalling the PE at chunk boundaries.
 - PSUM evictions go to the scalar engine (bias is all-zero for this problem;
   a nonzero-bias build falls back to DVE tensor_scalar_add).
 - x / w DMAs are batched 4 k-tiles per descriptor to cut sync-queue dispatch.
 - softmax reciprocal uses the 1-op DVE approx (~18 bits, plenty for bf16
   data) instead of the 3.3us InstReciprocal, removing the head-pair boundary
   stall that caused HAM clock-gate oscillation in SDPA.
"""

import sys
from contextlib import ExitStack

sys.path.insert(0, "/opt/trn_rl_repo")

import numpy as np
import ml_dtypes

import concourse.bass as bass
import concourse.bacc as bacc
import concourse.tile as tile
from concourse import mybir
from concourse import bass_utils

BF16 = ml_dtypes.bfloat16

T = 2048
C = 4096
H = 32
D = 128
R = 32
ROPE_BASE = 10000.0
N_CORES = 8
H_LOC = H // N_CORES          # 4 heads per core
KT = C // 128                 # 32 contraction tiles
KB = KT // 4                  # 8 batched-DMA groups of 4 k-tiles
NCH = T // 512                # 4 token chunks of 512
FQK = 2 * H_LOC               # 8 q/k feature tiles: f=2h -> q_h, f=2h+1 -> k_h
SCALE = 1.0 / float(np.sqrt(D))

# set by test.py to capture an NTFF profile; harness leaves False
TRACE = False
LAST_EXEC_NS = None
LAST_RESULTS = None

_CACHE = {}


def _build_program(bias_zero):
    """Build + compile the single-program SPMD Bass module (same code on all
    8 cores; per-core weights arrive via in_maps)."""
    nc = bacc.Bacc(
        "TRN2",
        target_bir_lowering=False,
        debug=False,
        enable_asserts=False,
        num_devices=N_CORES,
    )
    bf = mybir.dt.bfloat16
    f32 = mybir.dt.float32

    # [p, kt, t] layouts so one DMA covers several k-tiles
    xT_d = nc.dram_tensor("xT", (128, KT, T), bf, kind="ExternalInput").ap()
    wqkT_d = nc.dram_tensor("wqkT", (128, KT, FQK * 128), bf, kind="ExternalInput").ap()
    wvT_d = nc.dram_tensor("wvT", (128, KT, H_LOC * 128), bf, kind="ExternalInput").ap()
    wpT_d = nc.dram_tensor("wpT", (128, H_LOC, C), bf, kind="ExternalInput").ap()
    bqk_d = nc.dram_tensor("bqk", (128, FQK), f32, kind="ExternalInput").ap()
    cos_d = nc.dram_tensor("cosP", (R, T), bf, kind="ExternalInput").ap()
    sin_d = nc.dram_tensor("sinP", (R, T), bf, kind="ExternalInput").ap()
    mask_d = nc.dram_tensor("maskP", (4, 128, 512), bf, kind="ExternalInput").ap()
    maskb_d = nc.dram_tensor("maskB", (4, 128, 512), bf, kind="ExternalInput").ap()
    ident_d = nc.dram_tensor("identP", (128, 128), bf, kind="ExternalInput").ap()
    out_d = nc.dram_tensor("out", (T, C), bf, kind="ExternalOutput").ap()

    with tile.TileContext(nc) as tc:
        _emit(nc, tc, xT_d, wqkT_d, wvT_d, wpT_d, bqk_d, cos_d, sin_d, mask_d,
              maskb_d, ident_d, out_d, bias_zero)

    nc.compile()
    return nc


def _emit(nc, tc, xT_d, wqkT_d, wvT_d, wpT_d, bqk_d, cos_d, sin_d, mask_d,
          maskb_d, ident_d, out_d, bias_zero):
    bf = mybir.dt.bfloat16
    f32 = mybir.dt.float32

    # ---- persistent SBUF tensors (created before the working pools so that
    # pool release order stays LIFO: pools close first, singles after) -------
    frees = []

    def single(shape, dtype, name):
        t, fr = tc.tile(shape, dtype, name=name)
        frees.append(fr)
        return t

    qkT = single([128, FQK, T], bf, "qkT")          # Q^T/K^T: [d, f, t]
    vN = single([128, T // 128, H_LOC * 128], bf, "vN")  # V: [t%128, t//128, dv]
    yT = single([128, H_LOC, T], bf, "yT")          # normalized O^T per head
    ones = single([128, 128], bf, "ones")
    nc.vector.memset(ones, 1.0)
    # constants ride the gpsimd (SWDGE) queues so the sync queues can start
    # streaming the first x/w tiles immediately at kernel start
    bqk_sb = single([128, FQK], f32, "bqk_sb")
    if not bias_zero:
        nc.gpsimd.dma_start(out=bqk_sb, in_=bqk_d)
    cos_sb = single([R, T], bf, "cos_sb")
    sin_sb = single([R, T], bf, "sin_sb")
    nc.gpsimd.dma_start(out=cos_sb, in_=cos_d)
    nc.gpsimd.dma_start(out=sin_sb, in_=sin_d)
    # diagonal causal mask tiles: mask_r[jj, ii] = 1.0 if ii >= jj + 128*r
    maskt = single([128, 4, 512], bf, "maskt")
    for r in range(4):
        nc.gpsimd.dma_start(out=maskt[:, r, :], in_=mask_d[r])
    masks = [maskt[:, r, :] for r in range(4)]
    # additive variant (-30000 where masked) for chunk 0, where EVERY tile is
    # masked: folded into the score PSUM by an identity-lhsT matmul so the
    # DVE never sits on chunk 0's critical exp->mask->PV chain
    maskbt = single([128, 4, 512], bf, "maskbt")
    for r in range(4):
        nc.gpsimd.dma_start(out=maskbt[:, r, :], in_=maskb_d[r])
    ident = single([128, 128], bf, "ident")
    nc.gpsimd.dma_start(out=ident, in_=ident_d)
    # v weights are chunk-independent: keep resident instead of re-streaming
    wv_res = single([128, KT, 512], bf, "wv_res")

    with ExitStack() as ctx:
        xpool = ctx.enter_context(tc.tile_pool(name="xp", bufs=9))
        wqpool = ctx.enter_context(tc.tile_pool(name="wq", bufs=6))
        attpool = ctx.enter_context(tc.tile_pool(name="att", bufs=6))
        ropepool = ctx.enter_context(tc.tile_pool(name="rope", bufs=2))
        recippool = ctx.enter_context(tc.tile_pool(name="recip", bufs=2))
        stagepool = ctx.enter_context(tc.tile_pool(name="stage", bufs=3))
        wppool = ctx.enter_context(tc.tile_pool(name="wp", bufs=3))
        psum = ctx.enter_context(tc.tile_pool(name="psum", bufs=8, space="PSUM"))

        # PE warm-up: throwaway matmuls issued while the first weight and
        # activation DMAs are in flight, to lift the PE HAM clock gate to 8/8
        # before the real work arrives.
        warm = psum.tile([128, 128], f32, name="warm", tag="ps")
        for _ in range(96):
            nc.tensor.matmul(warm, lhsT=ones, rhs=ones, start=True, stop=True)

        def emit_rope(f, c):
            # q'[0:16]  = q[0:16]*cos - q[16:32]*sin
            # q'[16:32] = q[16:32]*cos + q[0:16]*sin
            # sin_sb rows 0..15 hold -sin, rows 16..31 hold +sin (host-folded).
            cs = slice(c * 512, (c + 1) * 512)
            rows = qkT[0:R, f, cs]
            swap = ropepool.tile([R, 512], bf, name=f"swap{f}_{c}", tag="swap")
            # gpsimd (SWDGE) queues: keeps these small SBUF->SBUF copies off
            # the sync queues that stream the main weight/activation tiles
            nc.gpsimd.dma_start(out=swap[0:16, :], in_=qkT[16:32, f, cs])
            nc.gpsimd.dma_start(out=swap[16:32, :], in_=qkT[0:16, f, cs])
            t1 = ropepool.tile([R, 512], bf, name=f"t1_{f}_{c}", tag="t1")
            nc.vector.tensor_mul(t1, swap, sin_sb[:, cs])
            nc.vector.tensor_mul(rows, rows, cos_sb[:, cs])
            nc.vector.tensor_add(rows, rows, t1)

        # ---- phase 1: QKV projection -------------------------------------
        # x batch tiles are shared by the A / B / V passes of a chunk.
        x_tiles = {}

        def emit_qk_group(c, g):
            # features g*4 .. g*4+3 in transposed layout (feature-major)
            pss = [
                psum.tile([128, 512], f32, name=f"qk_ps{c}_{g}_{i}", tag="ps")
                for i in range(4)
            ]
            for kb in range(KB):
                wb = wqpool.tile([128, 4, 512], bf, name=f"w{c}_{g}_{kb}", tag="wq")
                nc.sync.dma_start(
                    out=wb,
                    in_=wqkT_d[:, kb * 4 : (kb + 1) * 4, g * 512 : (g + 1) * 512],
                )
                xb = x_tiles[kb]
                for k4 in range(4):
                    for f in range(4):
                        nc.tensor.matmul(
                            pss[f],
                            lhsT=wb[:, k4, f * 128 : (f + 1) * 128],
                            rhs=xb[:, k4, :],
                            start=(kb == 0 and k4 == 0),
                            stop=(kb == KB - 1 and k4 == 3),
                        )
            last = c == NCH - 1
            for f in range(4):
                ff = g * 4 + f
                dst = qkT[:, ff, c * 512 : (c + 1) * 512]
                if not bias_zero:
                    nc.vector.tensor_scalar_add(dst, pss[f], bqk_sb[:, ff : ff + 1])
                elif last and g == 1:
                    # last chunk's B group: evict on DVE so the ACT queue is
                    # clear for chunk 0's first SDPA exps right after
                    nc.vector.tensor_copy(dst, pss[f])
                else:
                    nc.scalar.copy(dst, pss[f])
            if not last:
                for f in range(4):
                    emit_rope(g * 4 + f, c)
            # last chunk's rope is deferred into the SDPA phase (its k/q are
            # first consumed by the LAST SDPA chunk, ~200us later) so its DVE
            # ops don't queue ahead of chunk 0's epilogues

        def emit_v(c):
            # v in natural layout (token-major). Runs FIRST in each chunk: it
            # only streams x (weights are resident), so the qk weight streams
            # for A/B prefetch during its low-bandwidth window.
            psv = [
                psum.tile([128, 512], f32, name=f"v_ps{c}_{i}", tag="ps")
                for i in range(4)
            ]
            for kb in range(KB):
                xb = xpool.tile([128, 4, 512], bf, name=f"x{c}_{kb}", tag="x")
                nc.sync.dma_start(
                    out=xb,
                    in_=xT_d[:, kb * 4 : (kb + 1) * 4, c * 512 : (c + 1) * 512],
                )
                x_tiles[kb] = xb
                if c == 0:
                    nc.sync.dma_start(
                        out=wv_res[:, kb * 4 : (kb + 1) * 4, :],
                        in_=wvT_d[:, kb * 4 : (kb + 1) * 4, :],
                    )
                for k4 in range(4):
                    kt = kb * 4 + k4
                    for tt in range(4):
                        nc.tensor.matmul(
                            psv[tt],
                            lhsT=xb[:, k4, tt * 128 : (tt + 1) * 128],
                            rhs=wv_res[:, kt, :],
                            start=(kt == 0),
                            stop=(kt == KT - 1),
                        )
            for tt in range(4):
                nc.scalar.copy(vN[:, c * 4 + tt, :], psv[tt])

        for c in range(NCH):
            emit_v(c)
            emit_qk_group(c, 0)
            emit_qk_group(c, 1)

        # ---- phase 3+4: causal SDPA (chunk-outer, head-inner) with the
        # output projection for each finished chunk interleaved, keeping PE
        # fed while the scalar engine works on the next chunk's exps --------
        def proj_block(c, nchs):
            # output projection for the token tiles of chunk c, nch columns
            for nch in nchs:
                wp = wppool.tile([128, H_LOC, 512], bf, name=f"wp{c}_{nch}", tag="wp")
                nc.sync.dma_start(
                    out=wp, in_=wpT_d[:, :, nch * 512 : (nch + 1) * 512]
                )
                for tl in range(4):
                    tt = c * 4 + tl
                    pp = psum.tile([128, 512], f32, name=f"pp{c}_{nch}_{tl}", tag="ps")
                    for h in range(H_LOC):
                        nc.tensor.matmul(
                            pp,
                            lhsT=yT[:, h, tt * 128 : (tt + 1) * 128],
                            rhs=wp[:, h, :],
                            start=(h == 0),
                            stop=(h == H_LOC - 1),
                        )
                    st = stagepool.tile(
                        [128, 512], bf, name=f"st{c}_{nch}_{tl}", tag="st"
                    )
                    # alternate eviction between DVE and ACT: keeps the DVE
                    # FIFO short so the SDPA-critical mask/normalize ops
                    # behind it aren't delayed by bulk projection copies
                    if tl % 2 == 0:
                        nc.vector.tensor_copy(st, pp)
                    else:
                        nc.scalar.copy(st, pp)
                    nc.sync.dma_start(
                        out=out_d[
                            tt * 128 : (tt + 1) * 128, nch * 512 : (nch + 1) * 512
                        ],
                        in_=st,
                    )

        for c in range(NCH):
            if c == 1:
                # deferred rope for the last QKV chunk (consumed only by the
                # last SDPA chunk): its DVE/gpsimd work hides under c1's
                # matmul stream instead of stalling chunk 0's SDPA
                for f in range(FQK):
                    emit_rope(f, NCH - 1)
            njt = 4 * (c + 1)  # causal: key tiles 0 .. 4c+3
            for hp in range(H_LOC // 2):
                # previous chunk's projection matmuls are drip-fed INTO the
                # jt loop below so the PE always has independent work while
                # the exp->mask->PV chain of the current tiles is in flight
                pending = list(range(4 * hp, 4 * hp + 4)) if c > 0 else []
                hh = (2 * hp, 2 * hp + 1)
                o_ps = {}
                d_ps = {}
                # interleave two heads so PE always has an independent matmul
                # while the scalar engine works on the other head's exp
                for jt in range(njt):
                    for h in hh:
                        s_ps = psum.tile(
                            [128, 512], f32, name=f"s_ps{h}_{c}_{jt}", tag="ps"
                        )
                        nc.tensor.matmul(
                            s_ps,
                            lhsT=qkT[:, 2 * h + 1, jt * 128 : (jt + 1) * 128],
                            rhs=qkT[:, 2 * h, c * 512 : (c + 1) * 512],
                            start=True,
                            stop=(c != 0),
                        )
                        if c == 0:
                            # additive causal mask via identity matmul: keeps
                            # chunk 0 (every tile masked) off the DVE
                            nc.tensor.matmul(
                                s_ps,
                                lhsT=ident,
                                rhs=maskbt[:, jt, :],
                                start=False,
                                stop=True,
                            )
                        att = attpool.tile(
                            [128, 512], bf, name=f"att{h}_{c}_{jt}", tag="att"
                        )
                        nc.scalar.activation(
                            out=att,
                            in_=s_ps,
                            func=mybir.ActivationFunctionType.Exp,
                            scale=SCALE,
                        )
                        if jt == 0:
                            # allocate AFTER the first score tiles so a new
                            # head-pair's scores never wait on the previous
                            # pair's o/d slots (freed by the DVE normalize)
                            o_ps[h] = psum.tile(
                                [128, 512], f32, name=f"o_ps{h}_{c}", tag="ps"
                            )
                            d_ps[h] = psum.tile(
                                [128, 512], f32, name=f"d_ps{h}_{c}", tag="ps"
                            )
                        r = jt - 4 * c
                        if r >= 0 and c != 0:
                            nc.vector.tensor_mul(att, att, masks[r])
                        nc.tensor.matmul(
                            d_ps[h],
                            lhsT=ones,
                            rhs=att,
                            start=(jt == 0),
                            stop=(jt == njt - 1),
                        )
                        nc.tensor.matmul(
                            o_ps[h],
                            lhsT=vN[:, jt, h * 128 : (h + 1) * 128],
                            rhs=att,
                            start=(jt == 0),
                            stop=(jt == njt - 1),
                        )
                    if pending and h == hh[1]:
                        # one proj column per jt, starting immediately: the
                        # pending list always drains mid-loop so no burst of
                        # evictions lands on the ACT/DVE queues at the
                        # head-pair boundary right when the next pair's exp
                        # needs them
                        proj_block(c - 1, [pending.pop(0)])
                for nch in pending:
                    proj_block(c - 1, [nch])
                for h in hh:
                    rec = recippool.tile([128, 512], f32, name=f"rec{h}_{c}", tag="rec")
                    nc.vector.reciprocal_approx_fast(rec, d_ps[h])
                    nc.vector.tensor_mul(yT[:, h, c * 512 : (c + 1) * 512], o_ps[h], rec)
        proj_block(NCH - 1, range(C // 512))

    for fr in reversed(frees):
        fr()


def _rope_tables():
    theta = 1.0 / (ROPE_BASE ** (np.arange(0, R, 2, dtype=np.float64) / R))  # (16,)
    ang = np.outer(np.arange(T, dtype=np.float64), theta)  # (T, 16)
    cos = np.cos(ang).T  # (16, T)
    sin = np.sin(ang).T
    cosP = np.concatenate([cos, cos], axis=0)  # (32, T)
    sinP = np.concatenate([-sin, sin], axis=0)
    return np.ascontiguousarray(cosP).astype(BF16), np.ascontiguousarray(sinP).astype(BF16)


def _to_p_kt(a):
    """(rows, cols) -> (128, rows//128, cols): row r = [kt*128 + p]."""
    rows, cols = a.shape
    return np.ascontiguousarray(
        a.reshape(rows // 128, 128, cols).transpose(1, 0, 2)
    )


def kernel(x, w_attn, b_attn, w_proj, b_proj):
    x = np.asarray(x, dtype=np.float32)
    w_attn = np.asarray(w_attn, dtype=np.float32)
    b_attn = np.asarray(b_attn, dtype=np.float32)
    w_proj = np.asarray(w_proj, dtype=np.float32)
    b_proj = np.asarray(b_proj, dtype=np.float32)
    B = x.shape[0]
    assert (B, x.shape[1], x.shape[2]) == (1, T, C)

    bias_zero = bool(np.all(b_attn.reshape(H, 3, D)[:, :2, :] == 0.0))
    key = ("nc", bias_zero)
    if key not in _CACHE:
        _CACHE[key] = _build_program(bias_zero)
    nc = _CACHE[key]

    xT = _to_p_kt(x[0].T.astype(BF16))  # (128, 32, T)
    cosP, sinP = _rope_tables()
    # diagonal causal mask tiles: maskP[r, jj, ii] = 1.0 iff ii >= jj + 128*r
    jj = np.arange(128)[None, :, None]
    ii = np.arange(512)[None, None, :]
    rr = (128 * np.arange(4))[:, None, None]
    keep = ii >= jj + rr
    maskP = keep.astype(BF16)  # (4, 128, 512)
    maskB = np.where(keep, 0.0, -30000.0).astype(BF16)  # additive variant
    identP = np.eye(128, dtype=np.float32).astype(BF16)

    # w_attn rows per head h: [q (128), k (128), v (128)] at offset h*384
    wa = w_attn.reshape(H, 3, D, C)
    ba = b_attn.reshape(H, 3, D)
    in_maps = []
    for core in range(N_CORES):
        hs = range(core * H_LOC, (core + 1) * H_LOC)
        qk_rows = np.concatenate(
            [wa[h, t] for h in hs for t in (0, 1)], axis=0
        )  # (1024, C)  order: q_h0, k_h0, q_h1, k_h1, ...
        v_rows = np.concatenate([wa[h, 2] for h in hs], axis=0)  # (512, C)
        wqkT = _to_p_kt(qk_rows.T.astype(BF16))  # (128, 32, 1024)
        wvT = _to_p_kt(v_rows.T.astype(BF16))  # (128, 32, 512)
        wpT = _to_p_kt(
            w_proj[:, core * 512 : (core + 1) * 512].T.astype(BF16)
        )  # (128, 4, C)
        bqk = np.ascontiguousarray(
            np.stack([ba[h, t] for h in hs for t in (0, 1)], axis=0).T
        ).astype(np.float32)  # (128, 8)
        in_maps.append(
            dict(
                xT=xT, wqkT=wqkT, wvT=wvT, wpT=wpT, bqk=bqk,
                cosP=cosP, sinP=sinP, maskP=maskP, maskB=maskB, identP=identP,
            )
        )

    res = bass_utils.run_bass_kernel_spmd(
        nc, in_maps, core_ids=list(range(N_CORES)), trace=TRACE
    )
    global LAST_EXEC_NS, LAST_RESULTS
    LAST_EXEC_NS = res.exec_time_ns
    LAST_RESULTS = res

    out = np.zeros((T, C), dtype=np.float32)
    for core in range(N_CORES):
        out += res.results[core]["out"]

    # bias folds: q/k biases were applied on device; the v bias adds exactly
    # b_v to every y row (softmax rows sum to 1), so it folds into the output
    # bias along with b_proj.
    b_v = ba[:, 2, :].reshape(-1)  # (4096,)
    out += (w_proj @ b_v + b_proj)[None, :]
    return out.reshape(B, T, C).astype(np.float32)
